# revision 20
# baseline (speedup 1.0000x reference)
"""Trainium2 Bass kernel for nn_AttentionBlock (S=4096, H=1024, NH=2, DS=64).

Strategy (v6): sequence parallelism; bf16 projections with fp8 (e4m3)
attention, head-0 key packing, and four chunked fp8 AllGathers.

Each core owns 512 rows (queries AND keys). Head 0's mask depends only on
the KEY, so masked keys are dropped exactly: the host packs each core's
unmasked keys (<=263 of 512) into a 384-slot block of x^T; padding slots
carry a -1e30 exp bias so they contribute nothing. Head 1 uses all keys.

Host ships x^T / packed x^T / weights in bf16 in the exact SBUF layouts
plus vb = Wv@semb (the K-side semb constant cancels in softmax; the V-side
enters as a ones-row matmul into the V psum group). Projection PSUM
results are quantized to fp8 on vector-engine copy-out. K^T and V ship per
head-half as four chunked fp8 AllGathers triggered as soon as each
projection block finishes; no warm-up collective; all gpsimd DMAs are
issued before the first AG trigger because collective_compute blocks that
queue until the cross-core rendezvous completes.

QK, PV and the lsum run fp8: head 0 with plain contract-128 matmuls,
head 1 with DoubleRow contract-256 (A/B experiment for the PE rate).
exp(logits - 3.5) on the scalar engine stays inside e4m3 range (exact
softmax invariance). A ones block appended to the V tiles makes the lsum a
regular matmul whose PSUM result is already broadcast across partitions
for the normalize. ctx accumulates in PSUM, is normalized to bf16, and
out-projection + residual + LayerNorm run in bf16/f32 as in the reference.
"""

import math
import sys

sys.path.insert(0, "/opt/trn_rl_repo")

import numpy as np
import ml_dtypes

import concourse.bass as bass
import concourse.mybir as mybir
import concourse.tile as tile
from concourse import bacc
from concourse.bass_utils import run_bass_kernel_spmd

S, H, NH, DS = 4096, 1024, 2, 64
HD = H // NH            # 512
NC = 8                  # cores
SQ = S // NC            # 512 queries (and keys) per core
EPS = 1e-5
F32 = mybir.dt.float32
BF16 = mybir.dt.bfloat16
F8 = mybir.dt.float8e4
AF = mybir.ActivationFunctionType
ALU = mybir.AluOpType
PM = mybir.MatmulPerfMode.DoubleRow

KC = S // 128           # 32 global key chunks of 128 (head 1)
HC = H // 128           # 8 hidden chunks of 128
QB = SQ // 128          # 4 query chunks of 128
SC = 1.0 / math.sqrt(HD)               # exp input scale
MSHIFT = 3.5            # uniform logit shift: keeps exp() in fp8 range (exact)


def build_program(affine: bool, nk0: int):
    nj0 = nk0 // 128            # packed head-0 key chunks per core
    k0c = NC * nj0              # global packed head-0 key chunks
    k0g = NC * nk0              # global packed head-0 keys

    nc = bacc.Bacc("TRN2", target_bir_lowering=False, debug=False, num_devices=NC)

    # ---- DRAM I/O (host pre-arranged layouts) ----
    xTd = nc.dram_tensor("xTd", [128, HC, SQ], BF16, kind="ExternalInput")
    xTpd = nc.dram_tensor("xTpd", [128, HC, nk0], BF16, kind="ExternalInput")
    wqd = nc.dram_tensor("wqd", [128, HC, H], BF16, kind="ExternalInput")
    wkd = nc.dram_tensor("wkd", [128, HC, H], BF16, kind="ExternalInput")
    wvd = nc.dram_tensor("wvd", [128, HC, H], BF16, kind="ExternalInput")
    wod = nc.dram_tensor("wod", [128, HC, H], BF16, kind="ExternalInput")
    vbd = nc.dram_tensor("vbd", [1, H], BF16, kind="ExternalInput")
    xqd = nc.dram_tensor("xqd", [SQ, H], F32, kind="ExternalInput")
    mb0d = nc.dram_tensor("mb0d", [128, k0c], F32, kind="ExternalInput")
    onesrowd = nc.dram_tensor("onesrowd", [1, 128], BF16, kind="ExternalInput")
    if affine:
        lnw = nc.dram_tensor("lnw", [H], F32, kind="ExternalInput")
        lnb = nc.dram_tensor("lnb", [H], F32, kind="ExternalInput")
    out = nc.dram_tensor("out", [SQ, H], F32, kind="ExternalOutput")

    with tile.TileContext(nc) as tc:
        with (
            tc.tile_pool(name="consts", bufs=1) as consts,
            tc.tile_pool(name="persist", bufs=1) as persist,
            tc.tile_pool(name="rlp", bufs=1) as rlp,
            tc.tile_pool(name="dram", bufs=1, space="DRAM") as dram,
        ):
            # ---- constants (gpsimd DMAs all BEFORE any AG trigger) ----
            Af = consts.tile([128, k0c + 2], F32)   # 0:k0c mb0 | -M | eps
            mb0 = Af[:, 0:k0c]
            nc.gpsimd.dma_start(mb0, mb0d[:, :])
            zb_sb = Af[:, k0c:k0c + 1]
            nc.vector.memset(zb_sb, -MSHIFT)
            eps_sb = Af[:, k0c + 1:k0c + 2]
            nc.vector.memset(eps_sb, EPS)
            onesrow_sb = consts.tile([1, 128], BF16)
            nc.gpsimd.dma_start(onesrow_sb, onesrowd[:, :])
            vb_sb = consts.tile([1, H], BF16)
            nc.gpsimd.dma_start(vb_sb, vbd[:, :])
            if affine:
                lnw_b = consts.tile([128, H], F32)
                lnb_b = consts.tile([128, H], F32)
                nc.sync.dma_start(
                    lnw_b, bass.AP(tensor=lnw, offset=0, ap=[[0, 128], [1, H]]))
                nc.sync.dma_start(
                    lnb_b, bass.AP(tensor=lnb, offset=0, ap=[[0, 128], [1, H]]))

            # ---- persistent tiles ----
            xT_sb = persist.tile([128, HC, SQ], BF16)
            xTp_sb = persist.tile([128, HC, nk0], BF16)
            wq_sb = persist.tile([128, HC, H], BF16)
            wo_sb = persist.tile([128, HC, H], BF16)
            qT_sb = persist.tile([128, HC, SQ], F8)      # Q^T fp8 [d, q]
            kT0 = persist.tile([128, 4, k0g], F8)        # K^T h0 packed
            kT1 = persist.tile([128, 4, S], F8)          # K^T h1 full
            v0 = persist.tile([128, k0c, HD + 128], F8)  # V h0 packed + ones
            v1 = persist.tile([128, KC, HD + 128], F8)   # V h1 full + ones
            ctxT = persist.tile([128, HC, SQ], BF16)     # ctx^T/l bf16 [d, q]

            nc.vector.memset(v0[:, :, HD:HD + 128], 1.0)
            nc.vector.memset(v1[:, :, HD:HD + 128], 1.0)

            # DRAM scratch for collectives: K = h0 packed | h1 ; V likewise
            agw_k = 4 * nk0 + 4 * SQ
            agw_v = nj0 * HD + 4 * HD
            off_k1 = 4 * nk0
            off_v1 = nj0 * HD
            kvK = dram.tile([128, agw_k], F8, name="kvK")
            agKo = dram.tile([NC, 128, agw_k], F8, addr_space="Shared",
                             name="agKo")
            kvV = dram.tile([128, agw_v], F8, name="kvV")
            agVo = dram.tile([NC, 128, agw_v], F8, addr_space="Shared",
                             name="agVo")

            # ================= Stage 1: own-block projections =================
            # K projects plain x: the Wk@semb key-constant adds a per-query
            # offset to the logits which cancels in softmax. V gets Wv@semb
            # via a ones-row matmul with the host-computed vb row.
            with (
                tc.tile_pool(name="s1", bufs=1) as s1,
                tc.tile_pool(name="ps1", bufs=3, space="PSUM") as ps1,
            ):
                wk_sb = s1.tile([128, HC, H], BF16, tag="wk", name="wk_sb")
                wv_sb = s1.tile([128, HC, H], BF16, tag="wv", name="wv_sb")
                k_stage = s1.tile([128, agw_k], F8, tag="ks", name="k_stage")
                v_stage = s1.tile([128, agw_v], F8, tag="vs", name="v_stage")

                # loads; wk/xTp first (K^T h0 gates the first AllGather)
                nc.sync.dma_start(xTp_sb, xTpd[:, :, :])
                nc.scalar.dma_start(wk_sb[:, :, 0:HD], wkd[:, :, 0:HD])
                nc.gpsimd.dma_start(wk_sb[:, :, HD:H], wkd[:, :, HD:H])
                nc.sync.dma_start(xT_sb, xTd[:, :, :])
                nc.gpsimd.dma_start(wv_sb[:, :, 0:HD], wvd[:, :, 0:HD])
                nc.gpsimd.dma_start(wv_sb[:, :, HD:H], wvd[:, :, HD:H])
                nc.sync.dma_start(wq_sb[:, :, 0:HD], wqd[:, :, 0:HD])
                nc.sync.dma_start(wq_sb[:, :, HD:H], wqd[:, :, HD:H])

                # K^T: h0 packed then h1 full, one merged AllGather
                for h in range(2):
                    w = nk0 if h == 0 else SQ
                    src = xTp_sb if h == 0 else xT_sb
                    off = 0 if h == 0 else off_k1
                    for dci in range(4):
                        dc = 4 * h + dci
                        p = ps1.tile([128, SQ], F32, tag="pp", name=f"kp{dc}")
                        for hc in range(HC):
                            nc.tensor.matmul(
                                p[:, 0:w], wk_sb[:, hc, dc * 128:(dc + 1) * 128],
                                src[:, hc, :],
                                start=(hc == 0), stop=(hc == HC - 1))
                        nc.vector.tensor_copy(
                            k_stage[:, off + dci * w:off + (dci + 1) * w],
                            p[:, 0:w])
                nc.sync.dma_start(kvK[:, :], k_stage[:, :])
                nc.gpsimd.collective_compute(
                    "AllGather", mybir.AluOpType.bypass,
                    replica_groups=[list(range(NC))],
                    ins=[kvK.opt()], outs=[agKo.opt()],
                )

                # V: h0 d-half packed then h1 d-half full, one AllGather
                for hv in range(2):
                    njs = nj0 if hv == 0 else 4
                    src = xTp_sb if hv == 0 else xT_sb
                    off = 0 if hv == 0 else off_v1
                    for j in range(njs):
                        p = ps1.tile([128, HD], F32, tag="pp", name=f"vp{hv}_{j}")
                        nc.tensor.matmul(p[:], onesrow_sb[:, :],
                                         vb_sb[:, hv * HD:(hv + 1) * HD],
                                         start=True, stop=False)
                        for hc in range(HC):
                            nc.tensor.matmul(
                                p[:], src[:, hc, j * 128:(j + 1) * 128],
                                wv_sb[:, hc, hv * HD:(hv + 1) * HD],
                                start=False, stop=(hc == HC - 1))
                        nc.vector.tensor_copy(
                            v_stage[:, off + j * 512:off + (j + 1) * 512], p[:])
                nc.sync.dma_start(kvV[:, :], v_stage[:, :])
                nc.gpsimd.collective_compute(
                    "AllGather", mybir.AluOpType.bypass,
                    replica_groups=[list(range(NC))],
                    ins=[kvV.opt()], outs=[agVo.opt()],
                )
                # wo late on gpsimd: after the AG triggers, well before stage 3
                nc.gpsimd.dma_start(wo_sb, wod[:, :, :])

                # Q^T last (overlaps the AllGathers)
                for dc in range(HC):
                    p = ps1.tile([128, SQ], F32, tag="pp", name=f"qp{dc}")
                    for hc in range(HC):
                        nc.tensor.matmul(
                            p[:], wq_sb[:, hc, dc * 128:(dc + 1) * 128],
                            xT_sb[:, hc, :],
                            start=(hc == 0), stop=(hc == HC - 1))
                    nc.vector.tensor_copy(qT_sb[:, dc, :], p[:])

            # ============ Stage 2: gathered K/V + fp8 attention ==============
            with (
                tc.tile_pool(name="attn", bufs=1) as attn,
                tc.tile_pool(name="ps_s", bufs=3, space="PSUM") as ps_s,
                tc.tile_pool(name="ps_c", bufs=1, space="PSUM") as ps_c,
                tc.tile_pool(name="ps_l", bufs=1, space="PSUM") as ps_l,
            ):
                # copies from agout: kT split scalar/sync by parity (2 queues
                # drain the K payload ~2x faster so QK is not data-starved);
                # v0 on gpsimd, v1 on sync (after kT)
                kq = [nc.scalar, nc.sync, nc.gpsimd]
                for c in range(NC):
                    kq[c % 3].dma_start(
                        kT0[:, :, c * nk0:(c + 1) * nk0],
                        agKo[c][:, 0:off_k1].rearrange("p (a b) -> p a b", a=4))
                for c in range(NC):
                    kq[(c + 2) % 3].dma_start(
                        kT1[:, :, c * SQ:(c + 1) * SQ],
                        agKo[c][:, off_k1:agw_k].rearrange("p (a b) -> p a b", a=4))
                for c in range(NC):
                    nc.gpsimd.dma_start(
                        v0[:, c * nj0:(c + 1) * nj0, 0:HD],
                        agVo[c][:, 0:off_v1].rearrange("p (a b) -> p a b", a=nj0))
                    nc.sync.dma_start(
                        v1[:, 4 * c:4 * c + 4, 0:HD],
                        agVo[c][:, off_v1:agw_v].rearrange("p (a b) -> p a b", a=4))

                kcs = [k0c, KC]
                kts = [kT0, kT1]
                vts = [v0, v1]
                PTs = [attn.tile([128, kcs[h], SQ], F8, tag=f"PT{h}",
                                 name=f"PT{h}") for h in range(2)]

                # QK + exp for both heads first (exp on scalar is the floor)
                for h in range(2):
                    for kc in range(kcs[h]):
                        ps = ps_s.tile([128, SQ], F32, tag="st", name=f"st{h}_{kc}")
                        for dcp in range(2):
                            nc.tensor.matmul(
                                ps[:],
                                kts[h][:, 2 * dcp:2 * dcp + 2,
                                       kc * 128:(kc + 1) * 128],
                                qT_sb[:, 4 * h + 2 * dcp:4 * h + 2 * dcp + 2, :],
                                start=(dcp == 0), stop=(dcp == 1), perf_mode=PM)
                        bias_ap = mb0[:, kc:kc + 1] if h == 0 else zb_sb
                        nc.scalar.activation(PTs[h][:, kc, :], ps[:],
                                             AF.Exp, bias=bias_ap, scale=SC)

                # PV + lsum per head (DR over kc pairs), ctx in PSUM
                for h in range(2):
                    vt = vts[h]
                    ctx_ps = ps_c.tile([128, 4, SQ], F32, tag="ctx", name=f"ctx{h}")
                    lsum = ps_l.tile([128, SQ], F32, tag="ls", name=f"lsum{h}")
                    kcp_n = kcs[h] // 2
                    for kcp in range(kcp_n):
                        nc.tensor.matmul(
                            lsum[:],
                            vt[:, 2 * kcp:2 * kcp + 2, HD:HD + 128],
                            PTs[h][:, 2 * kcp:2 * kcp + 2, :],
                            start=(kcp == 0), stop=(kcp == kcp_n - 1),
                            perf_mode=PM, skip_group_check=True)
                        for dv in range(4):
                            nc.tensor.matmul(
                                ctx_ps[:, dv, :],
                                vt[:, 2 * kcp:2 * kcp + 2,
                                   dv * 128:(dv + 1) * 128],
                                PTs[h][:, 2 * kcp:2 * kcp + 2, :],
                                start=(kcp == 0), stop=(kcp == kcp_n - 1),
                                perf_mode=PM, skip_group_check=True)
                    # normalize: lsum is already broadcast across partitions
                    rl_b = rlp.tile([128, SQ], F32, tag="rlb", name=f"rlb{h}")
                    nc.vector.reciprocal(rl_b[:], lsum[:])
                    for dv in range(4):
                        nc.vector.tensor_mul(ctxT[:, 4 * h + dv, :],
                                             ctx_ps[:, dv, :], rl_b[:])

            # ---- Stage 3: bf16 out-proj + residual + LayerNorm ----
            with (
                tc.tile_pool(name="s4", bufs=2) as s4,
                tc.tile_pool(name="ps4", bufs=2, space="PSUM") as ps4,
            ):
                for qb in range(QB):
                    xq_f = s4.tile([128, H], F32, tag="xqf", name=f"xqf{qb}")
                    nc.sync.dma_start(xq_f, xqd[qb * 128:(qb + 1) * 128, :])
                    res_f = s4.tile([128, H], F32, tag="resf", name=f"resf{qb}")
                    for h2 in range(H // 512):
                        p = ps4.tile([128, 512], F32, tag="pout", name=f"po{qb}_{h2}")
                        for dc in range(HC):
                            nc.tensor.matmul(
                                p[:],
                                ctxT[:, dc, qb * 128:(qb + 1) * 128],
                                wo_sb[:, dc, h2 * 512:(h2 + 1) * 512],
                                start=(dc == 0), stop=(dc == HC - 1))
                        nc.vector.tensor_add(res_f[:, h2 * 512:(h2 + 1) * 512], p[:],
                                             xq_f[:, h2 * 512:(h2 + 1) * 512])
                    # LayerNorm via bn_stats
                    LS = s4.tile([128, 16], F32, tag="lns", name=f"lns{qb}")
                    for h2 in range(H // 512):
                        nc.vector.bn_stats(
                            LS[:, h2 * 6:(h2 + 1) * 6]
                            .rearrange("p (a b) -> p a b", a=1),
                            res_f[:, h2 * 512:(h2 + 1) * 512])
                    nc.vector.bn_aggr(LS[:, 12:14], LS[:, 0:12]
                                      .rearrange("p (a b) -> p a b", a=2))
                    nc.scalar.activation(LS[:, 14:15], LS[:, 13:14], AF.Sqrt,
                                         bias=eps_sb)
                    nc.vector.reciprocal(LS[:, 15:16], LS[:, 14:15])
                    fin = s4.tile([128, H], F32, tag="fin", name=f"fin{qb}")
                    for h2 in range(H // 512):
                        sl = slice(h2 * 512, (h2 + 1) * 512)
                        nc.vector.tensor_scalar(fin[:, sl], res_f[:, sl],
                                                LS[:, 12:13], LS[:, 15:16],
                                                ALU.subtract, ALU.mult)
                        if affine:
                            nc.vector.tensor_mul(fin[:, sl], fin[:, sl], lnw_b[:, sl])
                            nc.vector.tensor_add(fin[:, sl], fin[:, sl], lnb_b[:, sl])
                        nc.sync.dma_start(out[qb * 128:(qb + 1) * 128, sl],
                                          fin[:, sl])

    nc.compile()
    return nc


_CACHED_NC = {}


def _get_nc(affine: bool, nk0: int):
    key = (affine, nk0)
    if key not in _CACHED_NC:
        _CACHED_NC[key] = build_program(affine, nk0)
    return _CACHED_NC[key]


def _pack_T(a):
    """[R, C] f32 -> [128, R//128, C] bf16 with [p, rc, c] = a[rc*128+p, c]."""
    R, C = a.shape
    t = np.asarray(a, np.float32).reshape(R // 128, 128, C)
    return np.ascontiguousarray(t.transpose(1, 0, 2)).astype(ml_dtypes.bfloat16)


def _prep_inputs(inputs, static_data, base_mask, Wq, Wk, Wv, Wo, Ws, bs, ln_w, ln_b):
    f32 = np.float32
    bf = ml_dtypes.bfloat16
    x = np.asarray(inputs, f32)
    mask = np.asarray(base_mask, bool)
    semb = np.asarray(Ws, f32) @ np.asarray(static_data, f32) + np.asarray(bs, f32)
    vb = np.asarray(Wv, f32) @ semb
    affine = not (np.all(np.asarray(ln_w, f32) == 1.0)
                  and np.all(np.asarray(ln_b, f32) == 0.0))

    idxs = [np.where(mask[c * SQ:(c + 1) * SQ])[0] for c in range(NC)]
    maxn = max(len(ix) for ix in idxs)
    nk0 = 384 if maxn <= 384 else 512
    k0c = NC * (nk0 // 128)

    # packed head-0 exp bias: -MSHIFT for real keys, -1e30 for padding
    mb0 = np.full((NC, nk0), -1e30, f32)
    for c in range(NC):
        mb0[c, :len(idxs[c])] = -MSHIFT
    mb0 = mb0.reshape(k0c, 128).T

    common = {
        "wqd": _pack_T(np.asarray(Wq, f32).T),   # [din, dout] packed
        "wkd": _pack_T(np.asarray(Wk, f32).T),
        "wvd": _pack_T(np.asarray(Wv, f32).T),
        "wod": _pack_T(np.asarray(Wo, f32).T),
        "vbd": np.ascontiguousarray(vb.reshape(1, H)).astype(bf),
        "mb0d": np.ascontiguousarray(mb0),
        "onesrowd": np.ones((1, 128), bf),
    }
    if affine:
        common["lnw"] = np.ascontiguousarray(ln_w, f32)
        common["lnb"] = np.ascontiguousarray(ln_b, f32)
    in_maps = []
    for c in range(NC):
        rows = slice(c * SQ, (c + 1) * SQ)
        m = dict(common)
        m["xTd"] = _pack_T(x[rows].T)
        xp = np.zeros((nk0, H), f32)
        xp[:len(idxs[c])] = x[rows][idxs[c]]
        m["xTpd"] = _pack_T(xp.T)
        m["xqd"] = np.ascontiguousarray(x[rows])
        in_maps.append(m)
    return in_maps, affine, nk0


def kernel_run(trace=False, **inputs):
    in_maps, affine, nk0 = _prep_inputs(**inputs)
    nc = _get_nc(affine, nk0)
    res = run_bass_kernel_spmd(nc, in_maps, core_ids=list(range(NC)), trace=trace)
    outp = np.concatenate([res.results[c]["out"] for c in range(NC)], axis=0)
    return outp, res


def kernel(**inputs):
    outp, _ = kernel_run(trace=False, **inputs)
    return outp


# revision 21
# speedup vs baseline: 1.0660x; 1.0660x over previous
"""Trainium2 Bass kernel for nn_AttentionBlock (S=4096, H=1024, NH=2, DS=64).

Strategy (v6): sequence parallelism; bf16 projections with fp8 (e4m3)
attention, head-0 key packing, and four chunked fp8 AllGathers.

Each core owns 512 rows (queries AND keys). Head 0's mask depends only on
the KEY, so masked keys are dropped exactly: the host packs each core's
unmasked keys (<=263 of 512) into a 384-slot block of x^T; padding slots
carry a -1e30 exp bias so they contribute nothing. Head 1 uses all keys.

Host ships x^T / packed x^T / weights in bf16 in the exact SBUF layouts
plus vb = Wv@semb (the K-side semb constant cancels in softmax; the V-side
enters as a ones-row matmul into the V psum group). Projection PSUM
results are quantized to fp8 on vector-engine copy-out. K^T and V ship per
head-half as four chunked fp8 AllGathers triggered as soon as each
projection block finishes; no warm-up collective; all gpsimd DMAs are
issued before the first AG trigger because collective_compute blocks that
queue until the cross-core rendezvous completes.

QK, PV and the lsum run fp8: head 0 with plain contract-128 matmuls,
head 1 with DoubleRow contract-256 (A/B experiment for the PE rate).
exp(logits - 3.5) on the scalar engine stays inside e4m3 range (exact
softmax invariance). A ones block appended to the V tiles makes the lsum a
regular matmul whose PSUM result is already broadcast across partitions
for the normalize. ctx accumulates in PSUM, is normalized to bf16, and
out-projection + residual + LayerNorm run in bf16/f32 as in the reference.
"""

import math
import sys

sys.path.insert(0, "/opt/trn_rl_repo")

import numpy as np
import ml_dtypes

import concourse.bass as bass
import concourse.mybir as mybir
import concourse.tile as tile
from concourse import bacc
from concourse.bass_utils import run_bass_kernel_spmd

S, H, NH, DS = 4096, 1024, 2, 64
HD = H // NH            # 512
NC = 8                  # cores
SQ = S // NC            # 512 queries (and keys) per core
EPS = 1e-5
F32 = mybir.dt.float32
BF16 = mybir.dt.bfloat16
F8 = mybir.dt.float8e4
AF = mybir.ActivationFunctionType
ALU = mybir.AluOpType
PM = mybir.MatmulPerfMode.DoubleRow

KC = S // 128           # 32 global key chunks of 128 (head 1)
HC = H // 128           # 8 hidden chunks of 128
QB = SQ // 128          # 4 query chunks of 128
SC = 1.0 / math.sqrt(HD)               # exp input scale
MSHIFT = 3.5            # uniform logit shift: keeps exp() in fp8 range (exact)


def build_program(affine: bool, nk0: int):
    nj0 = nk0 // 128            # packed head-0 key chunks per core
    k0c = NC * nj0              # global packed head-0 key chunks
    k0g = NC * nk0              # global packed head-0 keys

    nc = bacc.Bacc("TRN2", target_bir_lowering=False, debug=False, num_devices=NC)

    # ---- DRAM I/O (host pre-arranged layouts) ----
    xTd = nc.dram_tensor("xTd", [128, HC, SQ], BF16, kind="ExternalInput")
    xTpd = nc.dram_tensor("xTpd", [128, HC, nk0], BF16, kind="ExternalInput")
    xT8d = nc.dram_tensor("xT8d", [128, HC, SQ], F8, kind="ExternalInput")
    xTp8d = nc.dram_tensor("xTp8d", [128, HC, nk0], F8, kind="ExternalInput")
    wqd = nc.dram_tensor("wqd", [128, HC, H], BF16, kind="ExternalInput")
    wkd = nc.dram_tensor("wkd", [128, HC, H], F8, kind="ExternalInput")
    wvd = nc.dram_tensor("wvd", [128, HC, H], BF16, kind="ExternalInput")
    wod = nc.dram_tensor("wod", [128, HC, H], BF16, kind="ExternalInput")
    vbd = nc.dram_tensor("vbd", [1, H], BF16, kind="ExternalInput")
    xqd = nc.dram_tensor("xqd", [SQ, H], F32, kind="ExternalInput")
    mb0d = nc.dram_tensor("mb0d", [128, k0c], F32, kind="ExternalInput")
    onesrowd = nc.dram_tensor("onesrowd", [1, 128], BF16, kind="ExternalInput")
    if affine:
        lnw = nc.dram_tensor("lnw", [H], F32, kind="ExternalInput")
        lnb = nc.dram_tensor("lnb", [H], F32, kind="ExternalInput")
    out = nc.dram_tensor("out", [SQ, H], F32, kind="ExternalOutput")

    with tile.TileContext(nc) as tc:
        with (
            tc.tile_pool(name="consts", bufs=1) as consts,
            tc.tile_pool(name="persist", bufs=1) as persist,
            tc.tile_pool(name="rlp", bufs=1) as rlp,
            tc.tile_pool(name="dram", bufs=1, space="DRAM") as dram,
        ):
            # ---- constants (gpsimd DMAs all BEFORE any AG trigger) ----
            Af = consts.tile([128, k0c + 2], F32)   # 0:k0c mb0 | -M | eps
            mb0 = Af[:, 0:k0c]
            nc.gpsimd.dma_start(mb0, mb0d[:, :])
            zb_sb = Af[:, k0c:k0c + 1]
            nc.vector.memset(zb_sb, -MSHIFT)
            eps_sb = Af[:, k0c + 1:k0c + 2]
            nc.vector.memset(eps_sb, EPS)
            onesrow_sb = consts.tile([1, 128], BF16)
            nc.gpsimd.dma_start(onesrow_sb, onesrowd[:, :])
            vb_sb = consts.tile([1, H], BF16)
            nc.gpsimd.dma_start(vb_sb, vbd[:, :])
            if affine:
                lnw_b = consts.tile([128, H], F32)
                lnb_b = consts.tile([128, H], F32)
                nc.sync.dma_start(
                    lnw_b, bass.AP(tensor=lnw, offset=0, ap=[[0, 128], [1, H]]))
                nc.sync.dma_start(
                    lnb_b, bass.AP(tensor=lnb, offset=0, ap=[[0, 128], [1, H]]))

            # ---- persistent tiles ----
            xT_sb = persist.tile([128, HC, SQ], BF16)
            xTp_sb = persist.tile([128, HC, nk0], BF16)
            xT8_sb = persist.tile([128, HC, SQ], F8)
            xTp8_sb = persist.tile([128, HC, nk0], F8)
            wq_sb = persist.tile([128, HC, H], BF16)
            wo_sb = persist.tile([128, HC, H], BF16)
            qT_sb = persist.tile([128, HC, SQ], F8)      # Q^T fp8 [d, q]
            kT0 = persist.tile([128, 4, k0g], F8)        # K^T h0 packed
            kT1 = persist.tile([128, 4, S], F8)          # K^T h1 full
            v0 = persist.tile([128, k0c, HD + 128], F8)  # V h0 packed + ones
            v1 = persist.tile([128, KC, HD + 128], F8)   # V h1 full + ones
            ctxT = persist.tile([128, HC, SQ], BF16)     # ctx^T/l bf16 [d, q]

            # DRAM scratch for collectives: K = h0 packed | h1 ; V likewise
            agw_k = 4 * nk0 + 4 * SQ
            agw_v = nj0 * HD + 4 * HD
            off_k1 = 4 * nk0
            off_v1 = nj0 * HD
            kvK = dram.tile([128, agw_k], F8, name="kvK")
            agKo = dram.tile([NC, 128, agw_k], F8, addr_space="Shared",
                             name="agKo")
            kvV = dram.tile([128, agw_v], F8, name="kvV")
            agVo = dram.tile([NC, 128, agw_v], F8, addr_space="Shared",
                             name="agVo")

            # ================= Stage 1: own-block projections =================
            # K projects plain x: the Wk@semb key-constant adds a per-query
            # offset to the logits which cancels in softmax. V gets Wv@semb
            # via a ones-row matmul with the host-computed vb row.
            with (
                tc.tile_pool(name="s1", bufs=1) as s1,
                tc.tile_pool(name="ps1", bufs=3, space="PSUM") as ps1,
            ):
                wk_sb = s1.tile([128, HC, H], F8, tag="wk", name="wk_sb")
                wv_sb = s1.tile([128, HC, H], BF16, tag="wv", name="wv_sb")
                k_stage = s1.tile([128, agw_k], F8, tag="ks", name="k_stage")
                v_stage = s1.tile([128, agw_v], F8, tag="vs", name="v_stage")

                # loads; wk(fp8)/xTp8 first (K^T gates the first AllGather)
                nc.sync.dma_start(xTp8_sb, xTp8d[:, :, :])
                nc.scalar.dma_start(wk_sb, wkd[:, :, :])
                nc.sync.dma_start(xT8_sb, xT8d[:, :, :])
                nc.gpsimd.dma_start(wv_sb[:, :, 0:HD], wvd[:, :, 0:HD])
                nc.gpsimd.dma_start(wv_sb[:, :, HD:H], wvd[:, :, HD:H])
                nc.sync.dma_start(xTp_sb, xTpd[:, :, :])
                nc.sync.dma_start(xT_sb, xTd[:, :, :])
                nc.scalar.dma_start(wq_sb[:, :, 0:HD], wqd[:, :, 0:HD])
                nc.scalar.dma_start(wq_sb[:, :, HD:H], wqd[:, :, HD:H])

                # K^T: fp8 DoubleRow, h0 packed then h1 full, one merged AG
                for h in range(2):
                    w = nk0 if h == 0 else SQ
                    src = xTp8_sb if h == 0 else xT8_sb
                    off = 0 if h == 0 else off_k1
                    for dci in range(4):
                        dc = 4 * h + dci
                        p = ps1.tile([128, SQ], F32, tag="pp", name=f"kp{dc}")
                        for i in range(4):
                            nc.tensor.matmul(
                                p[:, 0:w],
                                wk_sb[:, 2 * i:2 * i + 2, dc * 128:(dc + 1) * 128],
                                src[:, 2 * i:2 * i + 2, :],
                                start=(i == 0), stop=(i == 3), perf_mode=PM)
                        nc.vector.tensor_copy(
                            k_stage[:, off + dci * w:off + (dci + 1) * w],
                            p[:, 0:w])
                nc.sync.dma_start(kvK[:, :], k_stage[:, :])
                nc.gpsimd.collective_compute(
                    "AllGather", mybir.AluOpType.bypass,
                    replica_groups=[list(range(NC))],
                    ins=[kvK.opt()], outs=[agKo.opt()],
                )

                # V: h0 d-half packed then h1 d-half full, one AllGather
                for hv in range(2):
                    njs = nj0 if hv == 0 else 4
                    src = xTp_sb if hv == 0 else xT_sb
                    off = 0 if hv == 0 else off_v1
                    for j in range(njs):
                        p = ps1.tile([128, HD], F32, tag="pp", name=f"vp{hv}_{j}")
                        nc.tensor.matmul(p[:], onesrow_sb[:, :],
                                         vb_sb[:, hv * HD:(hv + 1) * HD],
                                         start=True, stop=False)
                        for hc in range(HC):
                            nc.tensor.matmul(
                                p[:], src[:, hc, j * 128:(j + 1) * 128],
                                wv_sb[:, hc, hv * HD:(hv + 1) * HD],
                                start=False, stop=(hc == HC - 1))
                        nc.vector.tensor_copy(
                            v_stage[:, off + j * 512:off + (j + 1) * 512], p[:])
                nc.sync.dma_start(kvV[:, :], v_stage[:, :])
                nc.gpsimd.collective_compute(
                    "AllGather", mybir.AluOpType.bypass,
                    replica_groups=[list(range(NC))],
                    ins=[kvV.opt()], outs=[agVo.opt()],
                )
                # wo late on gpsimd: after the AG triggers, well before stage 3
                nc.gpsimd.dma_start(wo_sb, wod[:, :, :])

                nc.vector.memset(v0[:, :, HD:HD + 128], 1.0)
                nc.vector.memset(v1[:, :, HD:HD + 128], 1.0)

                # Q^T last (overlaps the AllGathers)
                for dc in range(HC):
                    p = ps1.tile([128, SQ], F32, tag="pp", name=f"qp{dc}")
                    for hc in range(HC):
                        nc.tensor.matmul(
                            p[:], wq_sb[:, hc, dc * 128:(dc + 1) * 128],
                            xT_sb[:, hc, :],
                            start=(hc == 0), stop=(hc == HC - 1))
                    nc.vector.tensor_copy(qT_sb[:, dc, :], p[:])

            # ============ Stage 2: gathered K/V + fp8 attention ==============
            with (
                tc.tile_pool(name="attn", bufs=1) as attn,
                tc.tile_pool(name="ps_s", bufs=3, space="PSUM") as ps_s,
                tc.tile_pool(name="ps_c", bufs=1, space="PSUM") as ps_c,
                tc.tile_pool(name="ps_l", bufs=1, space="PSUM") as ps_l,
            ):
                # copies from agout: kT split scalar/sync by parity (2 queues
                # drain the K payload ~2x faster so QK is not data-starved);
                # v0 on gpsimd, v1 on sync (after kT)
                for c in range(NC):
                    eng = nc.scalar if c % 2 == 0 else nc.sync
                    eng.dma_start(
                        kT0[:, :, c * nk0:(c + 1) * nk0],
                        agKo[c][:, 0:off_k1].rearrange("p (a b) -> p a b", a=4))
                for c in range(NC):
                    eng = nc.sync if c % 2 == 0 else nc.scalar
                    eng.dma_start(
                        kT1[:, :, c * SQ:(c + 1) * SQ],
                        agKo[c][:, off_k1:agw_k].rearrange("p (a b) -> p a b", a=4))
                for c in range(NC):
                    nc.gpsimd.dma_start(
                        v0[:, c * nj0:(c + 1) * nj0, 0:HD],
                        agVo[c][:, 0:off_v1].rearrange("p (a b) -> p a b", a=nj0))
                    nc.sync.dma_start(
                        v1[:, 4 * c:4 * c + 4, 0:HD],
                        agVo[c][:, off_v1:agw_v].rearrange("p (a b) -> p a b", a=4))

                kcs = [k0c, KC]
                kts = [kT0, kT1]
                vts = [v0, v1]
                PTs = [attn.tile([128, kcs[h], SQ], F8, tag=f"PT{h}",
                                 name=f"PT{h}") for h in range(2)]

                # QK + exp for both heads first (exp on scalar is the floor)
                for h in range(2):
                    for kc in range(kcs[h]):
                        ps = ps_s.tile([128, SQ], F32, tag="st", name=f"st{h}_{kc}")
                        for dcp in range(2):
                            nc.tensor.matmul(
                                ps[:],
                                kts[h][:, 2 * dcp:2 * dcp + 2,
                                       kc * 128:(kc + 1) * 128],
                                qT_sb[:, 4 * h + 2 * dcp:4 * h + 2 * dcp + 2, :],
                                start=(dcp == 0), stop=(dcp == 1), perf_mode=PM)
                        bias_ap = mb0[:, kc:kc + 1] if h == 0 else zb_sb
                        nc.scalar.activation(PTs[h][:, kc, :], ps[:],
                                             AF.Exp, bias=bias_ap, scale=SC)

                # PV + lsum per head (DR over kc pairs), ctx in PSUM
                for h in range(2):
                    vt = vts[h]
                    ctx_ps = ps_c.tile([128, 4, SQ], F32, tag="ctx", name=f"ctx{h}")
                    lsum = ps_l.tile([128, SQ], F32, tag="ls", name=f"lsum{h}")
                    kcp_n = kcs[h] // 2
                    for kcp in range(kcp_n):
                        nc.tensor.matmul(
                            lsum[:],
                            vt[:, 2 * kcp:2 * kcp + 2, HD:HD + 128],
                            PTs[h][:, 2 * kcp:2 * kcp + 2, :],
                            start=(kcp == 0), stop=(kcp == kcp_n - 1),
                            perf_mode=PM, skip_group_check=True)
                        for dv in range(4):
                            nc.tensor.matmul(
                                ctx_ps[:, dv, :],
                                vt[:, 2 * kcp:2 * kcp + 2,
                                   dv * 128:(dv + 1) * 128],
                                PTs[h][:, 2 * kcp:2 * kcp + 2, :],
                                start=(kcp == 0), stop=(kcp == kcp_n - 1),
                                perf_mode=PM, skip_group_check=True)
                    # normalize: lsum is already broadcast across partitions
                    rl_b = rlp.tile([128, SQ], F32, tag="rlb", name=f"rlb{h}")
                    nc.vector.reciprocal(rl_b[:], lsum[:])
                    for dv in range(4):
                        nc.vector.tensor_mul(ctxT[:, 4 * h + dv, :],
                                             ctx_ps[:, dv, :], rl_b[:])

            # ---- Stage 3: bf16 out-proj + residual + LayerNorm ----
            with (
                tc.tile_pool(name="s4", bufs=2) as s4,
                tc.tile_pool(name="ps4", bufs=2, space="PSUM") as ps4,
            ):
                for qb in range(QB):
                    xq_f = s4.tile([128, H], F32, tag="xqf", name=f"xqf{qb}")
                    nc.sync.dma_start(xq_f, xqd[qb * 128:(qb + 1) * 128, :])
                    res_f = s4.tile([128, H], F32, tag="resf", name=f"resf{qb}")
                    for h2 in range(H // 512):
                        p = ps4.tile([128, 512], F32, tag="pout", name=f"po{qb}_{h2}")
                        for dc in range(HC):
                            nc.tensor.matmul(
                                p[:],
                                ctxT[:, dc, qb * 128:(qb + 1) * 128],
                                wo_sb[:, dc, h2 * 512:(h2 + 1) * 512],
                                start=(dc == 0), stop=(dc == HC - 1))
                        nc.vector.tensor_add(res_f[:, h2 * 512:(h2 + 1) * 512], p[:],
                                             xq_f[:, h2 * 512:(h2 + 1) * 512])
                    # LayerNorm via bn_stats
                    LS = s4.tile([128, 16], F32, tag="lns", name=f"lns{qb}")
                    for h2 in range(H // 512):
                        nc.vector.bn_stats(
                            LS[:, h2 * 6:(h2 + 1) * 6]
                            .rearrange("p (a b) -> p a b", a=1),
                            res_f[:, h2 * 512:(h2 + 1) * 512])
                    nc.vector.bn_aggr(LS[:, 12:14], LS[:, 0:12]
                                      .rearrange("p (a b) -> p a b", a=2))
                    nc.scalar.activation(LS[:, 14:15], LS[:, 13:14], AF.Sqrt,
                                         bias=eps_sb)
                    nc.vector.reciprocal(LS[:, 15:16], LS[:, 14:15])
                    fin = s4.tile([128, H], F32, tag="fin", name=f"fin{qb}")
                    for h2 in range(H // 512):
                        sl = slice(h2 * 512, (h2 + 1) * 512)
                        nc.vector.tensor_scalar(fin[:, sl], res_f[:, sl],
                                                LS[:, 12:13], LS[:, 15:16],
                                                ALU.subtract, ALU.mult)
                        if affine:
                            nc.vector.tensor_mul(fin[:, sl], fin[:, sl], lnw_b[:, sl])
                            nc.vector.tensor_add(fin[:, sl], fin[:, sl], lnb_b[:, sl])
                        nc.sync.dma_start(out[qb * 128:(qb + 1) * 128, sl],
                                          fin[:, sl])

    nc.compile()
    return nc


_CACHED_NC = {}


def _get_nc(affine: bool, nk0: int):
    key = (affine, nk0)
    if key not in _CACHED_NC:
        _CACHED_NC[key] = build_program(affine, nk0)
    return _CACHED_NC[key]


def _pack_T(a):
    """[R, C] f32 -> [128, R//128, C] bf16 with [p, rc, c] = a[rc*128+p, c]."""
    R, C = a.shape
    t = np.asarray(a, np.float32).reshape(R // 128, 128, C)
    return np.ascontiguousarray(t.transpose(1, 0, 2)).astype(ml_dtypes.bfloat16)


def _prep_inputs(inputs, static_data, base_mask, Wq, Wk, Wv, Wo, Ws, bs, ln_w, ln_b):
    f32 = np.float32
    bf = ml_dtypes.bfloat16
    x = np.asarray(inputs, f32)
    mask = np.asarray(base_mask, bool)
    semb = np.asarray(Ws, f32) @ np.asarray(static_data, f32) + np.asarray(bs, f32)
    vb = np.asarray(Wv, f32) @ semb
    affine = not (np.all(np.asarray(ln_w, f32) == 1.0)
                  and np.all(np.asarray(ln_b, f32) == 0.0))

    idxs = [np.where(mask[c * SQ:(c + 1) * SQ])[0] for c in range(NC)]
    maxn = max(len(ix) for ix in idxs)
    nk0 = 384 if maxn <= 384 else 512
    k0c = NC * (nk0 // 128)

    # packed head-0 exp bias: -MSHIFT for real keys, -1e30 for padding
    mb0 = np.full((NC, nk0), -1e30, f32)
    for c in range(NC):
        mb0[c, :len(idxs[c])] = -MSHIFT
    mb0 = mb0.reshape(k0c, 128).T

    common = {
        "wqd": _pack_T(np.asarray(Wq, f32).T),   # [din, dout] packed
        "wkd": _pack_T(np.asarray(Wk, f32).T).astype(ml_dtypes.float8_e4m3fn),
        "wvd": _pack_T(np.asarray(Wv, f32).T),
        "wod": _pack_T(np.asarray(Wo, f32).T),
        "vbd": np.ascontiguousarray(vb.reshape(1, H)).astype(bf),
        "mb0d": np.ascontiguousarray(mb0),
        "onesrowd": np.ones((1, 128), bf),
    }
    if affine:
        common["lnw"] = np.ascontiguousarray(ln_w, f32)
        common["lnb"] = np.ascontiguousarray(ln_b, f32)
    in_maps = []
    for c in range(NC):
        rows = slice(c * SQ, (c + 1) * SQ)
        m = dict(common)
        m["xTd"] = _pack_T(x[rows].T)
        xp = np.zeros((nk0, H), f32)
        xp[:len(idxs[c])] = x[rows][idxs[c]]
        m["xTpd"] = _pack_T(xp.T)
        m["xT8d"] = m["xTd"].astype(ml_dtypes.float8_e4m3fn)
        m["xTp8d"] = m["xTpd"].astype(ml_dtypes.float8_e4m3fn)
        m["xqd"] = np.ascontiguousarray(x[rows])
        in_maps.append(m)
    return in_maps, affine, nk0


def kernel_run(trace=False, **inputs):
    in_maps, affine, nk0 = _prep_inputs(**inputs)
    nc = _get_nc(affine, nk0)
    res = run_bass_kernel_spmd(nc, in_maps, core_ids=list(range(NC)), trace=trace)
    outp = np.concatenate([res.results[c]["out"] for c in range(NC)], axis=0)
    return outp, res


def kernel(**inputs):
    outp, _ = kernel_run(trace=False, **inputs)
    return outp
